# revision 9
# baseline (speedup 1.0000x reference)
"""TRN2 Bass kernel: 2-bit-quantized linear  y = x @ (levels[idx] * scale).T + bias.

Sharding: column-parallel over 8 NeuronCores - each core owns OUT_F/8 output
features (its slice of weight_indices / weight_scales / bias); x is replicated.

fp8 DoubleRow path (default):
  The 4-level codebook levels[0..3] is rescaled by a single global alpha
  (found by scanning one octave) so that all 4 values of alpha*levels round
  to fp8-e4m3 grid points with tiny relative error (~0.3% for these levels).
  Weights become W8[o,i] = e4m3(alpha*levels[idx[o,i]]) - an exact per-element
  recoding of the 2-bit index tensor - and the per-row scale is applied at
  PSUM drain as scale[o]/alpha (per-partition vector), bias fused likewise.

  x^T is quantized as xh = e4m3(x), and for the first N_LO of the 16
  256-row k-pairs additionally xl = e4m3(x - xh), giving a hi/lo split that
  restores most of the activation precision where corrected.  The moving
  stream is [16 hi pairs ; N_LO lo pairs] and the stationary W8 pair-tile is
  simply reused for the lo pairs, so the whole contraction is one PSUM
  accumulation chain of (16+N_LO) DoubleRow matmuls (fp8 runs at 0.5
  cycles/moving-row: 2 stacked K=128 contractions per instruction).

  Per 2048-token super-chunk, per 128-feature o-tile: 27 pair matmuls x 4
  chunk x 2 half slots accumulate into four [128,512] PSUM banks, then a
  ScalarE activation fuses scale+bias on drain (fp32 out), exactly like the
  fp16 baseline.  W8 (6.3MB fp8) stays SBUF-resident; xq streams.

The fp16 path (previous baseline) is kept for fallback/testing.
"""

import numpy as np
import ml_dtypes

import concourse.bass as bass
import concourse.bacc as bacc
import concourse.tile as tile
import concourse.mybir as mybir
from concourse.bass_utils import run_bass_kernel_spmd

AF = mybir.ActivationFunctionType
ALU = mybir.AluOpType
DT = mybir.dt

NCORES = 8

# Problem sizes (hardcoded per contract).
B, S, IN_F, OUT_F = 4, 1024, 4096, 12288
T_TOKENS = B * S
O_SHARD = OUT_F // NCORES

BF16 = ml_dtypes.bfloat16
F8 = mybir.dt.np(mybir.dt.float8e4)  # ml_dtypes.float8_e4m3 (TRN flavor)

N_LO = 10  # of the 16 k-pairs, how many get an fp8 lo-correction stream


# --------------------------------------------------------------------------
# fp8 DoubleRow program
# --------------------------------------------------------------------------
def build_program_fp8(
    *,
    in_f: int,
    t_tokens: int,
    o_shard: int,
    n_lo: int,
    sc_tokens: int = 1024,
    tc_size: int = 512,
    first_block_ots: int = 4,
    block_ots: int = 2,
    xq_extra_bufs: int | None = None,
    out_bufs: int = 6,
):
    """Single-core Bass/Tile program (SPMD across cores), fp8 DoubleRow."""
    assert in_f % 256 == 0 and o_shard % 128 == 0
    kp = in_f // 256          # stationary k-pairs
    n_pairs = kp + n_lo       # moving k-pairs (hi + lo)
    n_ot = o_shard // 128
    assert t_tokens % sc_tokens == 0 and sc_tokens % tc_size == 0
    assert tc_size % 256 == 0
    n_sc = t_tokens // sc_tokens
    n_ch = sc_tokens // tc_size
    n_h = tc_size // 256
    assert first_block_ots * n_ch <= 8 and block_ots * n_ch <= 4
    if xq_extra_bufs is None:
        # full next-super-chunk prefetch
        xq_extra_bufs = n_pairs + 2

    nc = bacc.Bacc("TRN2", target_bir_lowering=False, debug=False)

    xq_d = nc.dram_tensor(
        "xq", [n_pairs, 2, 128, t_tokens], DT.float8e4, kind="ExternalInput"
    )
    w8_d = nc.dram_tensor(
        "w8", [kp, 2, 128, o_shard], DT.float8e4, kind="ExternalInput"
    )
    scl_d = nc.dram_tensor("scl", [128, n_ot], DT.float32, kind="ExternalInput")
    bsv_d = nc.dram_tensor("bsv", [128, n_ot], DT.float32, kind="ExternalInput")
    yt_d = nc.dram_tensor("yt", [o_shard, t_tokens], DT.float32, kind="ExternalOutput")

    DR = mybir.MatmulPerfMode.DoubleRow

    with tile.TileContext(nc) as tc:
        with (
            tc.tile_pool(name="const", bufs=1) as cpool,
            tc.tile_pool(name="w8p", bufs=kp) as w8p,
            tc.tile_pool(name="xqp", bufs=n_pairs + xq_extra_bufs) as xqp,
            tc.tile_pool(name="outp", bufs=out_bufs) as outp,
            tc.tile_pool(name="ps", bufs=8, space=bass.MemorySpace.PSUM) as psp,
        ):
            scl_t = cpool.tile([128, n_ot], DT.float32, tag="scl")
            nc.sync.dma_start(scl_t[:], scl_d[:])
            bsv_t = cpool.tile([128, n_ot], DT.float32, tag="bsv")
            nc.sync.dma_start(bsv_t[:], bsv_d[:])

            # Stationary fp8 W^T pair-tiles, resident for the whole kernel.
            # DMA loads are interleaved with the first super-chunk's xq pair
            # loads below (pair-ordered) so the PE can start at pair 0 ASAP.
            w8_tiles = [
                w8p.tile([128, 2, o_shard], DT.float8e4, tag="w8", name="w8t")
                for _ in range(kp)
            ]
            w8_loaded = [False] * kp

            def load_w8(p):
                if not w8_loaded[p]:
                    # split early pairs across DMA engines for low latency
                    nsp = 4 if p < 2 else 1
                    osz = o_shard // nsp
                    for j in range(2):
                        for s in range(nsp):
                            nc.sync.dma_start(
                                w8_tiles[p][:, j, s * osz : (s + 1) * osz],
                                w8_d[p, j, :, s * osz : (s + 1) * osz],
                            )
                    w8_loaded[p] = True

            def mm_block(ots, xq_tiles, sc):
                """One PSUM accumulation block over the given o-tiles."""
                t0 = sc * sc_tokens
                pss = {
                    ot: [
                        psp.tile([128, tc_size], DT.float32, tag="ps", name="ps")
                        for _ in range(n_ch)
                    ]
                    for ot in ots
                }
                for p in range(n_pairs):
                    for ot in ots:
                        lhsT = w8_tiles[p if p < kp else p - kp][
                            :, :, ot * 128 : (ot + 1) * 128
                        ]
                        for c in range(n_ch):
                            for h in range(n_h):
                                off = c * tc_size + h * 256
                                # HW start=True zeroes the whole PSUM bank, so
                                # only the first co-located chain may issue it
                                # (verified by probe_dr.py y5/y6).
                                nc.tensor.matmul(
                                    pss[ot][c][:, h * 256 : (h + 1) * 256],
                                    lhsT,
                                    xq_tiles[p][:, :, off : off + 256],
                                    start=(p == 0 and h == 0),
                                    stop=(p == n_pairs - 1),
                                    perf_mode=DR,
                                    skip_group_check=True,
                                )
                for ot in ots:
                    for c in range(n_ch):
                        out_t = outp.tile([128, tc_size], DT.float32, tag="out")
                        nc.scalar.activation(
                            out_t[:],
                            pss[ot][c][:],
                            AF.Identity,
                            bias=bsv_t[:, ot : ot + 1],
                            scale=scl_t[:, ot : ot + 1],
                        )
                        nc.scalar.dma_start(
                            yt_d[
                                ot * 128 : (ot + 1) * 128,
                                t0 + c * tc_size : t0 + (c + 1) * tc_size,
                            ],
                            out_t[:],
                        )

            for sc in range(n_sc):
                t0 = sc * sc_tokens
                xq_tiles = []
                for p in range(n_pairs):
                    if sc == 0 and p < kp:
                        load_w8(p)
                    xt = xqp.tile([128, 2, sc_tokens], DT.float8e4, tag="xq")
                    nsp = 2 if (sc == 0 and p < 2) else 1
                    tsz = sc_tokens // nsp
                    for j in range(2):
                        for s in range(nsp):
                            nc.sync.dma_start(
                                xt[:, j, s * tsz : (s + 1) * tsz],
                                xq_d[p, j, :, t0 + s * tsz : t0 + (s + 1) * tsz],
                            )
                    xq_tiles.append(xt)

                # First block after cold start gets more o-tiles (more PSUM
                # banks, more PE work per pair) so streaming covers the ramp;
                # later blocks stay small so drains double-buffer.
                bo = first_block_ots if sc == 0 else block_ots
                ot0 = 0
                while ot0 < n_ot:
                    if sc > 0 or ot0 >= first_block_ots:
                        bo = block_ots
                    mm_block(tuple(range(ot0, min(ot0 + bo, n_ot))), xq_tiles, sc)
                    ot0 += bo

    nc.compile()
    return nc


def pick_alpha(levels: np.ndarray, n_grid: int = 8192):
    """Global rescale so all 4 codebook values land near e4m3 grid points."""
    lv = np.asarray(levels, dtype=np.float64)
    alphas = np.exp2(np.linspace(0.0, 1.0, n_grid, endpoint=False))
    v = np.outer(alphas, lv)
    q = v.astype(np.float32).astype(F8).astype(np.float64)
    w2 = lv**2
    rel = np.divide(q - v, v, out=np.zeros_like(v), where=v != 0)
    err = (rel**2 * w2).sum(axis=1) / max(w2.sum(), 1e-30)
    i = int(err.argmin())
    return float(alphas[i])


def make_in_maps_fp8(x, levels, weight_indices, weight_scales, bias, *, n_lo: int):
    t_tokens = x.shape[0] * x.shape[1]
    in_f = x.shape[2]
    o_shard = weight_indices.shape[0] // NCORES
    n_ot = o_shard // 128
    kp = in_f // 256

    alpha = pick_alpha(levels)
    codebook = (alpha * np.asarray(levels, dtype=np.float64)).astype(
        np.float32
    ).astype(F8)

    # moving stream: hi pairs (all) + lo pairs (first n_lo)
    x2 = np.asarray(x, dtype=np.float32).reshape(t_tokens, in_f)
    xt = np.ascontiguousarray(x2.T)  # [K, T] f32
    xh8 = xt.astype(F8)
    xq = np.empty((kp + n_lo, 2, 128, t_tokens), dtype=F8)
    xq[:kp] = xh8.reshape(kp, 2, 128, t_tokens)
    if n_lo:
        xl = (xt[: n_lo * 256] - xh8[: n_lo * 256].astype(np.float32)).astype(F8)
        xq[kp:] = xl.reshape(n_lo, 2, 128, t_tokens)

    idx = np.asarray(weight_indices)
    w8_full = codebook[idx]  # [OUT_F, K] fp8 (exact recode of the 2-bit tensor)

    in_maps = []
    for c in range(NCORES):
        o0, o1 = c * o_shard, (c + 1) * o_shard
        wt = np.ascontiguousarray(w8_full[o0:o1].view(np.uint8).T).view(F8)
        w8 = wt.reshape(kp, 2, 128, o_shard)
        scl = np.ascontiguousarray(
            (np.asarray(weight_scales[o0:o1], dtype=np.float64) / alpha)
            .astype(np.float32)
            .reshape(n_ot, 128)
            .T
        )
        bsv = np.ascontiguousarray(
            np.asarray(bias[o0:o1], dtype=np.float32).reshape(n_ot, 128).T
        )
        in_maps.append({"xq": xq, "w8": w8, "scl": scl, "bsv": bsv})
    return in_maps


# --------------------------------------------------------------------------
# fp16 path (previous baseline, kept as fallback)
# --------------------------------------------------------------------------
def build_program(
    *,
    in_f: int,
    t_tokens: int,
    o_shard: int,
    mode: str = "fp16",  # "fp16" | "f32r" | "bf16"
    tc_size: int = 512,
    o_cache_tiles: int | None = None,
    x_extra_bufs: int | None = None,
    out_bufs: int | None = None,
    gps_stripe: int = 0,
    ramp_groups: int = 8,
):
    assert in_f % 128 == 0 and o_shard % 128 == 0 and t_tokens % tc_size == 0
    kt = in_f // 128
    n_ot = o_shard // 128
    if o_cache_tiles is None:
        o_cache_tiles = n_ot if mode in ("bf16", "fp16") else max(1, n_ot // 2)
    assert n_ot % o_cache_tiles == 0
    n_phases = n_ot // o_cache_tiles
    n_tc = t_tokens // tc_size
    ow = o_cache_tiles * 128

    if mode == "bf16":
        x_dt = w_dt = i_dt = DT.bfloat16
        m_dt = DT.float32
    elif mode == "fp16":
        x_dt = w_dt = i_dt = m_dt = DT.float16
    else:  # f32r
        x_dt, w_dt = DT.float32r, DT.float32r
        i_dt = DT.float16
        m_dt = DT.float32

    if x_extra_bufs is None:
        x_extra_bufs = 6 if mode == "f32r" else 12
    if out_bufs is None:
        out_bufs = 4 if mode == "f32r" else 6

    nc = bacc.Bacc("TRN2", target_bir_lowering=False, debug=False)

    xt_d = nc.dram_tensor("xt", [in_f, t_tokens], x_dt, kind="ExternalInput")
    idx_d = nc.dram_tensor("idx", [in_f, o_shard], i_dt, kind="ExternalInput")
    coef_d = nc.dram_tensor("coef", [128, 4], DT.float32, kind="ExternalInput")
    scl_d = nc.dram_tensor("scl", [128, n_ot], DT.float32, kind="ExternalInput")
    bsv_d = nc.dram_tensor("bsv", [128, n_ot], DT.float32, kind="ExternalInput")
    yt_d = nc.dram_tensor("yt", [o_shard, t_tokens], DT.float32, kind="ExternalOutput")

    with tile.TileContext(nc) as tc:
        with (
            tc.tile_pool(name="const", bufs=1) as cpool,
            tc.tile_pool(name="wt", bufs=kt) as wtp,
            tc.tile_pool(name="idxp", bufs=3) as idxp,
            tc.tile_pool(name="tmp", bufs=2) as tmpp,
            tc.tile_pool(name="xtp", bufs=kt + x_extra_bufs) as xtp,
            tc.tile_pool(name="outp", bufs=out_bufs) as outp,
            tc.tile_pool(name="ps", bufs=8, space=bass.MemorySpace.PSUM) as psp,
        ):
            coef_t = cpool.tile([128, 4], DT.float32, tag="coef")
            nc.sync.dma_start(coef_t[:], coef_d[:])
            scl_t = cpool.tile([128, n_ot], DT.float32, tag="scl")
            nc.sync.dma_start(scl_t[:], scl_d[:])
            bsv_t = cpool.tile([128, n_ot], DT.float32, tag="bsv")
            nc.sync.dma_start(bsv_t[:], bsv_d[:])

            beta = coef_t[:, 0:1]
            gamma = coef_t[:, 1:2]
            c1 = coef_t[:, 2:3]
            c0 = coef_t[:, 3:4]

            def dequant_ktile(k, ph):
                it = idxp.tile([128, ow], i_dt, tag="it")
                nc.sync.dma_start(
                    it[:], idx_d[k * 128 : (k + 1) * 128, ph * ow : (ph + 1) * ow]
                )
                eng = (
                    nc.gpsimd
                    if (gps_stripe and k % gps_stripe == gps_stripe - 1)
                    else nc.vector
                )
                sq = tmpp.tile([128, ow], m_dt, tag="sq")
                nc.scalar.activation(sq[:], it[:], AF.Square)
                hh = tmpp.tile([128, ow], m_dt, tag="hh")
                eng.tensor_scalar(hh[:], it[:], c1, c0, op0=ALU.mult, op1=ALU.add)
                qq = tmpp.tile([128, ow], m_dt, tag="qq")
                eng.scalar_tensor_tensor(
                    qq[:], it[:], beta, sq[:], op0=ALU.add, op1=ALU.mult
                )
                wt = wtp.tile([128, ow], w_dt, tag="wt")
                eng.scalar_tensor_tensor(
                    wt[:], qq[:], gamma, hh[:], op0=ALU.mult, op1=ALU.add
                )
                return wt

            def load_chunk(tci):
                xts = []
                for k in range(kt):
                    xt_t = xtp.tile([128, tc_size], x_dt, tag="xt")
                    nc.sync.dma_start(
                        xt_t[:],
                        xt_d[
                            k * 128 : (k + 1) * 128,
                            tci * tc_size : (tci + 1) * tc_size,
                        ],
                    )
                    xts.append(xt_t)
                return xts

            def drain_store(ps, og, tci):
                out_t = outp.tile([128, tc_size], DT.float32, tag="out")
                nc.scalar.activation(
                    out_t[:],
                    ps[:],
                    AF.Identity,
                    bias=bsv_t[:, og : og + 1],
                    scale=scl_t[:, og : og + 1],
                )
                nc.scalar.dma_start(
                    yt_d[
                        og * 128 : (og + 1) * 128,
                        tci * tc_size : (tci + 1) * tc_size,
                    ],
                    out_t[:],
                )

            def mm_group(wt_tiles, xts, ot, tci, ph):
                ps = psp.tile([128, tc_size], DT.float32, tag="ps")
                for k in range(kt):
                    nc.tensor.matmul(
                        ps[:],
                        wt_tiles[k][:, ot * 128 : (ot + 1) * 128],
                        xts[k][:],
                        start=(k == 0),
                        stop=(k == kt - 1),
                    )
                drain_store(ps, ph * o_cache_tiles + ot, tci)

            for ph in range(n_phases):
                wt_tiles = [dequant_ktile(k, ph) for k in range(kt)]

                for tci in range(n_tc):
                    xts = load_chunk(tci)
                    first = tci == 0
                    if first and ramp_groups:
                        ra = list(range(min(ramp_groups, o_cache_tiles)))
                        pss = {
                            ot: psp.tile(
                                [128, tc_size], DT.float32, tag="ps", name="ps"
                            )
                            for ot in ra
                        }
                        for k in range(kt):
                            for ot in ra:
                                nc.tensor.matmul(
                                    pss[ot][:],
                                    wt_tiles[k][:, ot * 128 : (ot + 1) * 128],
                                    xts[k][:],
                                    start=(k == 0),
                                    stop=(k == kt - 1),
                                )
                        for ot in ra:
                            drain_store(pss[ot], ph * o_cache_tiles + ot, tci)
                        rest = range(len(ra), o_cache_tiles)
                    else:
                        rest = range(o_cache_tiles)
                    for ot in rest:
                        mm_group(wt_tiles, xts, ot, tci, ph)

    nc.compile()
    return nc


def poly_coeffs(levels: np.ndarray):
    lv = np.asarray(levels, dtype=np.float64)
    v = np.vander(np.arange(4.0), 4, increasing=True)
    c0, c1, c2, c3 = np.linalg.solve(v, lv)
    if abs(c3) < 1e-30:
        gamma = 1e-30
        beta = c2 / gamma
    else:
        gamma = c3
        beta = c2 / c3
    return float(beta), float(gamma), float(c1), float(c0)


def _np_dt(mode):
    return {"bf16": BF16, "fp16": np.float16}.get(mode, np.float32)


def make_in_maps(x, levels, weight_indices, weight_scales, bias, *, mode: str):
    t_tokens = x.shape[0] * x.shape[1]
    in_f = x.shape[2]
    o_shard = weight_indices.shape[0] // NCORES
    n_ot = o_shard // 128

    x2 = np.asarray(x, dtype=np.float32).reshape(t_tokens, in_f)
    xt = np.ascontiguousarray(x2.T)
    xt = xt.astype(_np_dt(mode)) if mode in ("bf16", "fp16") else xt

    i_np = BF16 if mode == "bf16" else np.float16
    beta, gamma, c1, c0 = poly_coeffs(levels)
    coef = np.tile(np.array([beta, gamma, c1, c0], dtype=np.float32), (128, 1))

    in_maps = []
    for c in range(NCORES):
        o0, o1 = c * o_shard, (c + 1) * o_shard
        idx_t = np.ascontiguousarray(
            np.asarray(weight_indices[o0:o1], dtype=np.float32).T
        ).astype(i_np)
        scl = np.ascontiguousarray(
            np.asarray(weight_scales[o0:o1], dtype=np.float32).reshape(n_ot, 128).T
        )
        bsv = np.ascontiguousarray(
            np.asarray(bias[o0:o1], dtype=np.float32).reshape(n_ot, 128).T
        )
        in_maps.append({"xt": xt, "idx": idx_t, "coef": coef, "scl": scl, "bsv": bsv})
    return in_maps


_PROGRAM_CACHE: dict = {}


def _get_program(mode: str):
    if mode not in _PROGRAM_CACHE:
        if mode == "fp8dr":
            _PROGRAM_CACHE[mode] = build_program_fp8(
                in_f=IN_F, t_tokens=T_TOKENS, o_shard=O_SHARD, n_lo=N_LO
            )
        else:
            _PROGRAM_CACHE[mode] = build_program(
                in_f=IN_F, t_tokens=T_TOKENS, o_shard=O_SHARD, mode=mode
            )
    return _PROGRAM_CACHE[mode]


def run_on_cores(x, levels, weight_indices, weight_scales, bias, *, mode: str,
                 trace: bool = False):
    nc = _get_program(mode)
    if mode == "fp8dr":
        in_maps = make_in_maps_fp8(
            x, levels, weight_indices, weight_scales, bias, n_lo=N_LO
        )
    else:
        in_maps = make_in_maps(
            x, levels, weight_indices, weight_scales, bias, mode=mode
        )
    res = run_bass_kernel_spmd(
        nc, in_maps, core_ids=list(range(NCORES)), trace=trace
    )
    yt = np.concatenate([res.results[c]["yt"] for c in range(NCORES)], axis=0)
    y = np.ascontiguousarray(yt.T).reshape(B, S, OUT_F)
    return y, res


def kernel(x, levels, weight_indices, weight_scales, bias):
    y, _ = run_on_cores(x, levels, weight_indices, weight_scales, bias, mode="fp8dr")
    return y


# revision 15
# speedup vs baseline: 1.2428x; 1.2428x over previous
"""TRN2 Bass kernel: 2-bit-quantized linear  y = x @ (levels[idx] * scale).T + bias.

Sharding: column-parallel over 8 NeuronCores - each core owns OUT_F/8 output
features (its slice of weight_indices / weight_scales / bias); x is replicated.

fp8 DoubleRow path (default):
  The 4-level codebook levels[0..3] is rescaled by a single global alpha
  (found by scanning one octave) so that all 4 values of alpha*levels round
  to fp8-e4m3 grid points with tiny relative error (~0.3% for these levels).
  Weights become W8[o,i] = e4m3(alpha*levels[idx[o,i]]) - an exact per-element
  recoding of the 2-bit index tensor - and the per-row scale is applied at
  PSUM drain as scale[o]/alpha (per-partition vector), bias fused likewise.

  x^T is quantized as xh = e4m3(x), and for the first N_LO of the 16
  256-row k-pairs additionally xl = e4m3(x - xh), giving a hi/lo split that
  restores most of the activation precision where corrected.  The moving
  stream is [16 hi pairs ; N_LO lo pairs] and the stationary W8 pair-tile is
  simply reused for the lo pairs, so the whole contraction is one PSUM
  accumulation chain of (16+N_LO) DoubleRow matmuls (fp8 runs at 0.5
  cycles/moving-row: 2 stacked K=128 contractions per instruction).

  Per 2048-token super-chunk, per 128-feature o-tile: 27 pair matmuls x 4
  chunk x 2 half slots accumulate into four [128,512] PSUM banks, then a
  ScalarE activation fuses scale+bias on drain (fp32 out), exactly like the
  fp16 baseline.  W8 (6.3MB fp8) stays SBUF-resident; xq streams.

The fp16 path (previous baseline) is kept for fallback/testing.
"""

import numpy as np
import ml_dtypes

import concourse.bass as bass
import concourse.bacc as bacc
import concourse.tile as tile
import concourse.mybir as mybir
from concourse.bass_utils import run_bass_kernel_spmd

AF = mybir.ActivationFunctionType
ALU = mybir.AluOpType
DT = mybir.dt

NCORES = 8

# Problem sizes (hardcoded per contract).
B, S, IN_F, OUT_F = 4, 1024, 4096, 12288
T_TOKENS = B * S
O_SHARD = OUT_F // NCORES

BF16 = ml_dtypes.bfloat16
F8 = mybir.dt.np(mybir.dt.float8e4)  # ml_dtypes.float8_e4m3 (TRN flavor)

N_LO = 9  # of the 16 k-pairs, how many get an fp8 lo-correction stream


# --------------------------------------------------------------------------
# fp8 DoubleRow program
# --------------------------------------------------------------------------
def build_program_fp8(
    *,
    in_f: int,
    t_tokens: int,
    o_shard: int,
    n_lo: int,
    sc_tokens: int = 2048,
    tc_size: int = 512,
    xq_extra_bufs: int | None = None,
    out_bufs: int = 6,
):
    """Single-core Bass/Tile program (SPMD across cores), fp8 DoubleRow."""
    assert in_f % 256 == 0 and o_shard % 128 == 0
    kp = in_f // 256          # stationary k-pairs
    n_pairs = kp + n_lo       # moving k-pairs (hi + lo)
    n_ot = o_shard // 128
    assert t_tokens % sc_tokens == 0 and sc_tokens % tc_size == 0
    assert tc_size % 256 == 0
    n_sc = t_tokens // sc_tokens
    n_ch = sc_tokens // tc_size
    n_h = tc_size // 256
    n_og = (n_ot + 1) // 2
    if xq_extra_bufs is None:
        xq_extra_bufs = 5

    nc = bacc.Bacc("TRN2", target_bir_lowering=False, debug=False)

    xq_d = nc.dram_tensor(
        "xq", [n_pairs, 2, 128, t_tokens], DT.float8e4, kind="ExternalInput"
    )
    w8_d = nc.dram_tensor(
        "w8", [kp, 2, 128, o_shard], DT.float8e4, kind="ExternalInput"
    )
    scl_d = nc.dram_tensor("scl", [128, n_ot], DT.float32, kind="ExternalInput")
    bsv_d = nc.dram_tensor("bsv", [128, n_ot], DT.float32, kind="ExternalInput")
    yt_d = nc.dram_tensor("yt", [o_shard, t_tokens], DT.float32, kind="ExternalOutput")

    DR = mybir.MatmulPerfMode.DoubleRow

    with tile.TileContext(nc) as tc:
        with (
            tc.tile_pool(name="const", bufs=1) as cpool,
            tc.tile_pool(name="w8p", bufs=kp * n_og) as w8p,
            tc.tile_pool(name="xqp", bufs=n_pairs + xq_extra_bufs) as xqp,
            tc.tile_pool(name="outp", bufs=out_bufs) as outp,
            tc.tile_pool(name="ps", bufs=8, space=bass.MemorySpace.PSUM) as psp,
        ):
            scl_t = cpool.tile([128, n_ot], DT.float32, tag="scl")
            nc.sync.dma_start(scl_t[:], scl_d[:])
            bsv_t = cpool.tile([128, n_ot], DT.float32, tag="bsv")
            nc.sync.dma_start(bsv_t[:], bsv_d[:])

            # Stationary fp8 W^T pair-tiles, resident for the whole kernel.
            # DMA loads are interleaved with the first super-chunk's xq pair
            # loads below (pair-ordered) so the PE can start at pair 0 ASAP.
            # W8 split into per-o-tile-pair tiles [128, 2, 256] so the first
            # matmul block only depends on its own 1MB slice of weights.
            w8_tiles = [
                [
                    w8p.tile([128, 2, 256], DT.float8e4, tag="w8", name="w8t")
                    for _ in range(n_og)
                ]
                for _ in range(kp)
            ]

            def load_w8(p, og):
                for j in range(2):
                    nc.sync.dma_start(
                        w8_tiles[p][og][:, j, :],
                        w8_d[p, j, :, og * 256 : (og + 1) * 256],
                    )

            def mm_block(ots, xq_tiles, sc):
                """One PSUM accumulation block over the given o-tiles."""
                t0 = sc * sc_tokens
                pss = {
                    ot: [
                        psp.tile([128, tc_size], DT.float32, tag="ps", name="ps")
                        for _ in range(n_ch)
                    ]
                    for ot in ots
                }
                for p in range(n_pairs):
                    for ot in ots:
                        lhsT = w8_tiles[p if p < kp else p - kp][ot // 2][
                            :, :, (ot % 2) * 128 : (ot % 2 + 1) * 128
                        ]
                        for c in range(n_ch):
                            for h in range(n_h):
                                off = c * tc_size + h * 256
                                # HW start=True zeroes the whole PSUM bank, so
                                # only the first co-located chain may issue it
                                # (verified by probe_dr.py y5/y6).
                                nc.tensor.matmul(
                                    pss[ot][c][:, h * 256 : (h + 1) * 256],
                                    lhsT,
                                    xq_tiles[p][:, :, off : off + 256],
                                    start=(p == 0 and h == 0),
                                    stop=(p == n_pairs - 1),
                                    perf_mode=DR,
                                    skip_group_check=True,
                                )
                for ot in ots:
                    for c in range(n_ch):
                        out_t = outp.tile([128, tc_size], DT.float32, tag="out")
                        nc.scalar.activation(
                            out_t[:],
                            pss[ot][c][:],
                            AF.Identity,
                            bias=bsv_t[:, ot : ot + 1],
                            scale=scl_t[:, ot : ot + 1],
                        )
                        nc.scalar.dma_start(
                            yt_d[
                                ot * 128 : (ot + 1) * 128,
                                t0 + c * tc_size : t0 + (c + 1) * tc_size,
                            ],
                            out_t[:],
                        )

            for sc in range(n_sc):
                t0 = sc * sc_tokens
                xq_tiles = []
                for p in range(n_pairs):
                    if sc == 0 and p < kp:
                        load_w8(p, 0)  # first o-group weights + xq first
                    xt = xqp.tile([128, 2, sc_tokens], DT.float8e4, tag="xq")
                    nsp = 2 if (sc == 0 and p < 2) else 1
                    tsz = sc_tokens // nsp
                    for j in range(2):
                        for s in range(nsp):
                            nc.sync.dma_start(
                                xt[:, j, s * tsz : (s + 1) * tsz],
                                xq_d[p, j, :, t0 + s * tsz : t0 + (s + 1) * tsz],
                            )
                    xq_tiles.append(xt)
                if sc == 0:
                    for og in range(1, n_og):
                        for p in range(kp):
                            load_w8(p, og)

                if sc == 0 and n_ot % 2 == 0:
                    # Ramp: pair o-tiles (8 PSUM banks; 16 matmuls per pair,
                    # 8 consecutive per stationary) so PE pace per pair
                    # matches DMA delivery while xq/w8 stream in.
                    for og in range(0, n_ot, 2):
                        mm_block((og, og + 1), xq_tiles, sc)
                else:
                    for ot in range(n_ot):
                        mm_block((ot,), xq_tiles, sc)

    nc.compile()
    return nc


def pick_alpha(levels: np.ndarray, n_grid: int = 8192):
    """Global rescale so all 4 codebook values land near e4m3 grid points."""
    lv = np.asarray(levels, dtype=np.float64)
    alphas = np.exp2(np.linspace(0.0, 1.0, n_grid, endpoint=False))
    v = np.outer(alphas, lv)
    q = v.astype(np.float32).astype(F8).astype(np.float64)
    w2 = lv**2
    rel = np.divide(q - v, v, out=np.zeros_like(v), where=v != 0)
    err = (rel**2 * w2).sum(axis=1) / max(w2.sum(), 1e-30)
    i = int(err.argmin())
    return float(alphas[i])


def make_in_maps_fp8(x, levels, weight_indices, weight_scales, bias, *, n_lo: int):
    t_tokens = x.shape[0] * x.shape[1]
    in_f = x.shape[2]
    o_shard = weight_indices.shape[0] // NCORES
    n_ot = o_shard // 128
    kp = in_f // 256

    alpha = pick_alpha(levels)
    codebook = (alpha * np.asarray(levels, dtype=np.float64)).astype(
        np.float32
    ).astype(F8)

    # moving stream: hi pairs (all) + lo pairs (first n_lo)
    x2 = np.asarray(x, dtype=np.float32).reshape(t_tokens, in_f)
    xt = np.ascontiguousarray(x2.T)  # [K, T] f32
    xh8 = xt.astype(F8)
    xq = np.empty((kp + n_lo, 2, 128, t_tokens), dtype=F8)
    xq[:kp] = xh8.reshape(kp, 2, 128, t_tokens)
    if n_lo:
        xl = (xt[: n_lo * 256] - xh8[: n_lo * 256].astype(np.float32)).astype(F8)
        xq[kp:] = xl.reshape(n_lo, 2, 128, t_tokens)

    idx = np.asarray(weight_indices)
    w8_full = codebook[idx]  # [OUT_F, K] fp8 (exact recode of the 2-bit tensor)

    in_maps = []
    for c in range(NCORES):
        o0, o1 = c * o_shard, (c + 1) * o_shard
        wt = np.ascontiguousarray(w8_full[o0:o1].view(np.uint8).T).view(F8)
        w8 = wt.reshape(kp, 2, 128, o_shard)
        scl = np.ascontiguousarray(
            (np.asarray(weight_scales[o0:o1], dtype=np.float64) / alpha)
            .astype(np.float32)
            .reshape(n_ot, 128)
            .T
        )
        bsv = np.ascontiguousarray(
            np.asarray(bias[o0:o1], dtype=np.float32).reshape(n_ot, 128).T
        )
        in_maps.append({"xq": xq, "w8": w8, "scl": scl, "bsv": bsv})
    return in_maps


# --------------------------------------------------------------------------
# fp16 path (previous baseline, kept as fallback)
# --------------------------------------------------------------------------
def build_program(
    *,
    in_f: int,
    t_tokens: int,
    o_shard: int,
    mode: str = "fp16",  # "fp16" | "f32r" | "bf16"
    tc_size: int = 512,
    o_cache_tiles: int | None = None,
    x_extra_bufs: int | None = None,
    out_bufs: int | None = None,
    gps_stripe: int = 0,
    ramp_groups: int = 8,
):
    assert in_f % 128 == 0 and o_shard % 128 == 0 and t_tokens % tc_size == 0
    kt = in_f // 128
    n_ot = o_shard // 128
    if o_cache_tiles is None:
        o_cache_tiles = n_ot if mode in ("bf16", "fp16") else max(1, n_ot // 2)
    assert n_ot % o_cache_tiles == 0
    n_phases = n_ot // o_cache_tiles
    n_tc = t_tokens // tc_size
    ow = o_cache_tiles * 128

    if mode == "bf16":
        x_dt = w_dt = i_dt = DT.bfloat16
        m_dt = DT.float32
    elif mode == "fp16":
        x_dt = w_dt = i_dt = m_dt = DT.float16
    else:  # f32r
        x_dt, w_dt = DT.float32r, DT.float32r
        i_dt = DT.float16
        m_dt = DT.float32

    if x_extra_bufs is None:
        x_extra_bufs = 6 if mode == "f32r" else 12
    if out_bufs is None:
        out_bufs = 4 if mode == "f32r" else 6

    nc = bacc.Bacc("TRN2", target_bir_lowering=False, debug=False)

    xt_d = nc.dram_tensor("xt", [in_f, t_tokens], x_dt, kind="ExternalInput")
    idx_d = nc.dram_tensor("idx", [in_f, o_shard], i_dt, kind="ExternalInput")
    coef_d = nc.dram_tensor("coef", [128, 4], DT.float32, kind="ExternalInput")
    scl_d = nc.dram_tensor("scl", [128, n_ot], DT.float32, kind="ExternalInput")
    bsv_d = nc.dram_tensor("bsv", [128, n_ot], DT.float32, kind="ExternalInput")
    yt_d = nc.dram_tensor("yt", [o_shard, t_tokens], DT.float32, kind="ExternalOutput")

    with tile.TileContext(nc) as tc:
        with (
            tc.tile_pool(name="const", bufs=1) as cpool,
            tc.tile_pool(name="wt", bufs=kt) as wtp,
            tc.tile_pool(name="idxp", bufs=3) as idxp,
            tc.tile_pool(name="tmp", bufs=2) as tmpp,
            tc.tile_pool(name="xtp", bufs=kt + x_extra_bufs) as xtp,
            tc.tile_pool(name="outp", bufs=out_bufs) as outp,
            tc.tile_pool(name="ps", bufs=8, space=bass.MemorySpace.PSUM) as psp,
        ):
            coef_t = cpool.tile([128, 4], DT.float32, tag="coef")
            nc.sync.dma_start(coef_t[:], coef_d[:])
            scl_t = cpool.tile([128, n_ot], DT.float32, tag="scl")
            nc.sync.dma_start(scl_t[:], scl_d[:])
            bsv_t = cpool.tile([128, n_ot], DT.float32, tag="bsv")
            nc.sync.dma_start(bsv_t[:], bsv_d[:])

            beta = coef_t[:, 0:1]
            gamma = coef_t[:, 1:2]
            c1 = coef_t[:, 2:3]
            c0 = coef_t[:, 3:4]

            def dequant_ktile(k, ph):
                it = idxp.tile([128, ow], i_dt, tag="it")
                nc.sync.dma_start(
                    it[:], idx_d[k * 128 : (k + 1) * 128, ph * ow : (ph + 1) * ow]
                )
                eng = (
                    nc.gpsimd
                    if (gps_stripe and k % gps_stripe == gps_stripe - 1)
                    else nc.vector
                )
                sq = tmpp.tile([128, ow], m_dt, tag="sq")
                nc.scalar.activation(sq[:], it[:], AF.Square)
                hh = tmpp.tile([128, ow], m_dt, tag="hh")
                eng.tensor_scalar(hh[:], it[:], c1, c0, op0=ALU.mult, op1=ALU.add)
                qq = tmpp.tile([128, ow], m_dt, tag="qq")
                eng.scalar_tensor_tensor(
                    qq[:], it[:], beta, sq[:], op0=ALU.add, op1=ALU.mult
                )
                wt = wtp.tile([128, ow], w_dt, tag="wt")
                eng.scalar_tensor_tensor(
                    wt[:], qq[:], gamma, hh[:], op0=ALU.mult, op1=ALU.add
                )
                return wt

            def load_chunk(tci):
                xts = []
                for k in range(kt):
                    xt_t = xtp.tile([128, tc_size], x_dt, tag="xt")
                    nc.sync.dma_start(
                        xt_t[:],
                        xt_d[
                            k * 128 : (k + 1) * 128,
                            tci * tc_size : (tci + 1) * tc_size,
                        ],
                    )
                    xts.append(xt_t)
                return xts

            def drain_store(ps, og, tci):
                out_t = outp.tile([128, tc_size], DT.float32, tag="out")
                nc.scalar.activation(
                    out_t[:],
                    ps[:],
                    AF.Identity,
                    bias=bsv_t[:, og : og + 1],
                    scale=scl_t[:, og : og + 1],
                )
                nc.scalar.dma_start(
                    yt_d[
                        og * 128 : (og + 1) * 128,
                        tci * tc_size : (tci + 1) * tc_size,
                    ],
                    out_t[:],
                )

            def mm_group(wt_tiles, xts, ot, tci, ph):
                ps = psp.tile([128, tc_size], DT.float32, tag="ps")
                for k in range(kt):
                    nc.tensor.matmul(
                        ps[:],
                        wt_tiles[k][:, ot * 128 : (ot + 1) * 128],
                        xts[k][:],
                        start=(k == 0),
                        stop=(k == kt - 1),
                    )
                drain_store(ps, ph * o_cache_tiles + ot, tci)

            for ph in range(n_phases):
                wt_tiles = [dequant_ktile(k, ph) for k in range(kt)]

                for tci in range(n_tc):
                    xts = load_chunk(tci)
                    first = tci == 0
                    if first and ramp_groups:
                        ra = list(range(min(ramp_groups, o_cache_tiles)))
                        pss = {
                            ot: psp.tile(
                                [128, tc_size], DT.float32, tag="ps", name="ps"
                            )
                            for ot in ra
                        }
                        for k in range(kt):
                            for ot in ra:
                                nc.tensor.matmul(
                                    pss[ot][:],
                                    wt_tiles[k][:, ot * 128 : (ot + 1) * 128],
                                    xts[k][:],
                                    start=(k == 0),
                                    stop=(k == kt - 1),
                                )
                        for ot in ra:
                            drain_store(pss[ot], ph * o_cache_tiles + ot, tci)
                        rest = range(len(ra), o_cache_tiles)
                    else:
                        rest = range(o_cache_tiles)
                    for ot in rest:
                        mm_group(wt_tiles, xts, ot, tci, ph)

    nc.compile()
    return nc


def poly_coeffs(levels: np.ndarray):
    lv = np.asarray(levels, dtype=np.float64)
    v = np.vander(np.arange(4.0), 4, increasing=True)
    c0, c1, c2, c3 = np.linalg.solve(v, lv)
    if abs(c3) < 1e-30:
        gamma = 1e-30
        beta = c2 / gamma
    else:
        gamma = c3
        beta = c2 / c3
    return float(beta), float(gamma), float(c1), float(c0)


def _np_dt(mode):
    return {"bf16": BF16, "fp16": np.float16}.get(mode, np.float32)


def make_in_maps(x, levels, weight_indices, weight_scales, bias, *, mode: str):
    t_tokens = x.shape[0] * x.shape[1]
    in_f = x.shape[2]
    o_shard = weight_indices.shape[0] // NCORES
    n_ot = o_shard // 128

    x2 = np.asarray(x, dtype=np.float32).reshape(t_tokens, in_f)
    xt = np.ascontiguousarray(x2.T)
    xt = xt.astype(_np_dt(mode)) if mode in ("bf16", "fp16") else xt

    i_np = BF16 if mode == "bf16" else np.float16
    beta, gamma, c1, c0 = poly_coeffs(levels)
    coef = np.tile(np.array([beta, gamma, c1, c0], dtype=np.float32), (128, 1))

    in_maps = []
    for c in range(NCORES):
        o0, o1 = c * o_shard, (c + 1) * o_shard
        idx_t = np.ascontiguousarray(
            np.asarray(weight_indices[o0:o1], dtype=np.float32).T
        ).astype(i_np)
        scl = np.ascontiguousarray(
            np.asarray(weight_scales[o0:o1], dtype=np.float32).reshape(n_ot, 128).T
        )
        bsv = np.ascontiguousarray(
            np.asarray(bias[o0:o1], dtype=np.float32).reshape(n_ot, 128).T
        )
        in_maps.append({"xt": xt, "idx": idx_t, "coef": coef, "scl": scl, "bsv": bsv})
    return in_maps


_PROGRAM_CACHE: dict = {}


def _get_program(mode: str):
    if mode not in _PROGRAM_CACHE:
        if mode == "fp8dr":
            _PROGRAM_CACHE[mode] = build_program_fp8(
                in_f=IN_F, t_tokens=T_TOKENS, o_shard=O_SHARD, n_lo=N_LO
            )
        else:
            _PROGRAM_CACHE[mode] = build_program(
                in_f=IN_F, t_tokens=T_TOKENS, o_shard=O_SHARD, mode=mode
            )
    return _PROGRAM_CACHE[mode]


def run_on_cores(x, levels, weight_indices, weight_scales, bias, *, mode: str,
                 trace: bool = False):
    nc = _get_program(mode)
    if mode == "fp8dr":
        in_maps = make_in_maps_fp8(
            x, levels, weight_indices, weight_scales, bias, n_lo=N_LO
        )
    else:
        in_maps = make_in_maps(
            x, levels, weight_indices, weight_scales, bias, mode=mode
        )
    res = run_bass_kernel_spmd(
        nc, in_maps, core_ids=list(range(NCORES)), trace=trace
    )
    yt = np.concatenate([res.results[c]["yt"] for c in range(NCORES)], axis=0)
    y = np.ascontiguousarray(yt.T).reshape(B, S, OUT_F)
    return y, res


def kernel(x, levels, weight_indices, weight_scales, bias):
    y, _ = run_on_cores(x, levels, weight_indices, weight_scales, bias, mode="fp8dr")
    return y
